# revision 1
# baseline (speedup 1.0000x reference)
"""Gumbel-Sinkhorn (masked, 5 iterations) on Trainium2, data-parallel over 8 cores.

Math: the reference's masked log-domain Sinkhorn is equivalent, in the
probability domain, to classic Sinkhorn scaling of K = exp(masked_logits):

    v_0 = 1;   u_k = 1 / (K v_{k-1});   v_k = 1 / (K^T u_k)      (k = 1..5)
    out = K * (u_5 outer v_5) * exp(1e-6),  masked entries exactly 0.

Per core (64 samples), everything runs as PE matvecs with the sample matrix
as the stationary operand, batched DVE reciprocals across a cohort of
samples, and a final PE-broadcast + ACT/DVE materialization.

Host-side prep (cheap, O(B*A*T) numpy): mask logits to -1e4 (exp -> exact 0)
and build the per-sample transposed copy so both row- and column-phase
matvecs contract along the SBUF partition axis.
"""

import numpy as np

B, A, T = 512, 256, 256
NCORES = 8
BPC = B // NCORES          # samples per core
C = 16                     # cohort size (samples in lockstep)
ITERS = 5
MASKVAL = np.float32(-1e4)  # exp(-1e4) == 0.0 exactly in fp32
EPS = 1e-15                 # guards 1/0 on fully-masked rows/cols
OUT_SCALE = float(np.exp(np.float64(1e-6)))  # reference's exp(x + 1e-6)

_NC_CACHE = None


def _build_nc():
    import concourse.tile as tile
    from concourse import bacc, mybir

    f32 = mybir.dt.float32
    AF = mybir.ActivationFunctionType

    nc = bacc.Bacc()
    lg = nc.dram_tensor("lg", [BPC, A, T], f32, kind="ExternalInput")
    lgT = nc.dram_tensor("lgT", [BPC, T, A], f32, kind="ExternalInput")
    ident = nc.dram_tensor("ident", [128, 128], f32, kind="ExternalInput")
    # sel[k, b*128+m] = OUT_SCALE if k == b else 0: selector weights that turn
    # the [C,256] v-row tile into a per-sample [128,256] broadcast via PE.
    sel = nc.dram_tensor("sel", [C, C * 128], f32, kind="ExternalInput")
    out = nc.dram_tensor("out", [BPC, A, T], f32, kind="ExternalOutput")

    G = BPC // C  # number of cohorts
    SLAB = C * 512  # free elems per slab: per sample 2 halves x 256

    with tile.TileContext(nc) as tc:
        with (
            tc.tile_pool(name="e0p", bufs=2) as e0p,
            tc.tile_pool(name="etp", bufs=2) as etp,
            tc.tile_pool(name="uvp", bufs=10) as uvp,
            tc.tile_pool(name="wp", bufs=4) as wp,
            tc.tile_pool(name="vrowp", bufs=2) as vrowp,
            tc.tile_pool(name="constp", bufs=1) as constp,
            tc.tile_pool(name="psuv", bufs=2, space="PSUM") as psuv,
            tc.tile_pool(name="psbc", bufs=3, space="PSUM") as psbc,
            tc.tile_pool(name="psvr", bufs=1, space="PSUM") as psvr,
        ):
            ident_sb = constp.tile([128, 128], f32)
            nc.sync.dma_start(ident_sb[:], ident[:])
            sel_sb = constp.tile([C, C * 128], f32)
            nc.sync.dma_start(sel_sb[0:C, :], sel[:])
            v_ones = constp.tile([128, 2 * C], f32)
            nc.vector.memset(v_ones[:], 1.0)

            for g in range(G):
                s0 = g * C
                # ---- load + exp (masked logits -> K, K^T) ----
                e0 = e0p.tile([128, SLAB], f32, name="e0")
                et = etp.tile([128, SLAB], f32, name="et")
                src = lg[s0:s0 + C].rearrange("b (h p) j -> p b h j", p=128)
                nc.sync.dma_start(e0[:].rearrange("p (b h j) -> p b h j", h=2, j=256), src)
                srcT = lgT[s0:s0 + C].rearrange("b (h p) j -> p b h j", p=128)
                nc.sync.dma_start(et[:].rearrange("p (b h j) -> p b h j", h=2, j=256), srcT)
                nc.scalar.activation(e0[:], e0[:], AF.Exp)
                nc.scalar.activation(et[:], et[:], AF.Exp)

                # ---- 5 Sinkhorn iterations (scale vectors only) ----
                # layout of u/v tiles: [128, 2C], column h*C + b = half h of sample b
                v_cur = v_ones
                u_cur = None
                for it in range(ITERS):
                    ps_u = psuv.tile([128, 2 * C], f32, name="ps_u")
                    for b in range(C):
                        for ia in range(2):
                            for jt in range(2):
                                nc.tensor.matmul(
                                    ps_u[:, ia * C + b: ia * C + b + 1],
                                    lhsT=et[:, b * 512 + jt * 256 + ia * 128:
                                            b * 512 + jt * 256 + ia * 128 + 128],
                                    rhs=v_cur[:, jt * C + b: jt * C + b + 1],
                                    start=(jt == 0), stop=(jt == 1),
                                )
                    u_t = uvp.tile([128, 2 * C], f32, name="u_t")
                    nc.vector.tensor_scalar_max(u_t[:], ps_u[:], EPS)
                    u_cur = uvp.tile([128, 2 * C], f32, name="u_cur")
                    nc.vector.reciprocal(u_cur[:], u_t[:])

                    ps_v = psuv.tile([128, 2 * C], f32, name="ps_v")
                    for b in range(C):
                        for jt in range(2):
                            for ia in range(2):
                                nc.tensor.matmul(
                                    ps_v[:, jt * C + b: jt * C + b + 1],
                                    lhsT=e0[:, b * 512 + ia * 256 + jt * 128:
                                            b * 512 + ia * 256 + jt * 128 + 128],
                                    rhs=u_cur[:, ia * C + b: ia * C + b + 1],
                                    start=(ia == 0), stop=(ia == 1),
                                )
                    v_t = uvp.tile([128, 2 * C], f32, name="v_t")
                    nc.vector.tensor_scalar_max(v_t[:], ps_v[:], EPS)
                    v_cur = uvp.tile([128, 2 * C], f32, name="v_cur")
                    nc.vector.reciprocal(v_cur[:], v_t[:])

                # ---- materialize out = e0 * (u outer v) * OUT_SCALE ----
                # v columns -> rows (one PE transpose per half, whole cohort)
                ps_vr = psvr.tile([128, 256], f32, name="ps_vr")
                for jt in range(2):
                    nc.tensor.transpose(
                        ps_vr[0:C, jt * 128:(jt + 1) * 128],
                        v_cur[:, jt * C:(jt + 1) * C],
                        ident_sb[:],
                    )
                vrow = vrowp.tile([128, 256], f32, name="vrow")
                nc.vector.tensor_copy(vrow[0:C, :], ps_vr[0:C, :])

                for b in range(C):
                    ps_b = psbc.tile([128, 256], f32, name="ps_b")
                    # [128,256] per-sample broadcast of v-row, scaled by OUT_SCALE
                    nc.tensor.matmul(
                        ps_b[:], lhsT=sel_sb[0:C, b * 128:(b + 1) * 128],
                        rhs=vrow[0:C, :], start=True, stop=True,
                    )
                    for ia in range(2):
                        w = wp.tile([128, 256], f32, name="w")
                        nc.scalar.activation(
                            w[:], ps_b[:], AF.Copy,
                            scale=u_cur[:, ia * C + b: ia * C + b + 1],
                        )
                        sl = slice(b * 512 + ia * 256, b * 512 + (ia + 1) * 256)
                        nc.vector.tensor_mul(e0[:, sl], e0[:, sl], w[:])

                dst = out[s0:s0 + C].rearrange("b (h p) j -> p b h j", p=128)
                nc.sync.dma_start(dst, e0[:].rearrange("p (b h j) -> p b h j", h=2, j=256))

    nc.compile()
    return nc


def _get_nc():
    global _NC_CACHE
    if _NC_CACHE is None:
        _NC_CACHE = _build_nc()
    return _NC_CACHE


def _prep_in_maps(logits, free_agents_num, tasks_num):
    logits = np.asarray(logits, dtype=np.float32)
    free = np.asarray(free_agents_num).astype(np.int64)
    tasks = np.asarray(tasks_num).astype(np.int64)
    row_ok = np.arange(A, dtype=np.int64)[None, :] < free[:, None]   # [B, A]
    col_ok = np.arange(T, dtype=np.int64)[None, :] < tasks[:, None]  # [B, T]
    mask = row_ok[:, :, None] & col_ok[:, None, :]
    lgm = np.where(mask, logits, MASKVAL).astype(np.float32)
    lgmT = np.ascontiguousarray(lgm.transpose(0, 2, 1))
    ident = np.eye(128, dtype=np.float32)
    sel = np.zeros((C, C * 128), dtype=np.float32)
    for b in range(C):
        sel[b, b * 128:(b + 1) * 128] = OUT_SCALE
    return [
        {
            "lg": np.ascontiguousarray(lgm[c * BPC:(c + 1) * BPC]),
            "lgT": lgmT[c * BPC:(c + 1) * BPC],
            "ident": ident,
            "sel": sel,
        }
        for c in range(NCORES)
    ]


def _run(logits, free_agents_num, tasks_num, **spmd_kwargs):
    from concourse.bass_utils import run_bass_kernel_spmd

    in_maps = _prep_in_maps(logits, free_agents_num, tasks_num)
    res = run_bass_kernel_spmd(
        _get_nc(), in_maps, core_ids=list(range(NCORES)), **spmd_kwargs
    )
    out = np.concatenate([r["out"] for r in res.results], axis=0)
    return out, res


def kernel(logits, free_agents_num, tasks_num):
    out, _ = _run(logits, free_agents_num, tasks_num)
    return out



# revision 18
# speedup vs baseline: 1.3930x; 1.3930x over previous
"""Gumbel-Sinkhorn (masked, 5 iterations) on Trainium2, data-parallel over 8 cores.

Math: the reference's masked log-domain Sinkhorn equals, in probability
domain, classic Sinkhorn scaling of K = exp(masked_logits):

    v_0 = 1;   u_k = 1 / (K v_{k-1} + eps);   v_k = 1 / (K^T u_k + eps)
    out = K * (u_5 outer v_5) * exp(1e-6),  masked entries exactly 0.

HBM traffic is the roofline (memory regime): masked logits are loaded once
and the output stored once (32 MB/core).  K^T is built on-chip with PE
transposes instead of a second 16 MB HBM load.  Engines run their queues
in order, so the kernel is software-pipelined over cohort pairs with a
3-stage overlap per cycle: loads are hoisted 2 pairs ahead (SP queue),
prep (exp + K^T build, ACT/PE) 1 pair ahead, and pair p-1's output
materialization is interleaved into pair p's Sinkhorn phases.  The eps
guard is seeded into PSUM by a rank-1 PE matmul so each phase needs only a
single DVE reciprocal; the final u-scaling runs on the otherwise idle
GpSimd engine (SBUF-only there: no PSUM port).
"""

import numpy as np

B, A, T = 512, 256, 256
NCORES = 8
BPC = B // NCORES          # samples per core
C = 8                      # cohort size (samples in lockstep)
ITERS = 5
MASKVAL = np.float32(-1e4)  # exp(-1e4) == 0.0 exactly in fp32
EPS = 1e-15                 # guards 1/0 on fully-masked rows/cols
OUT_SCALE = float(np.exp(np.float64(1e-6)))  # reference's exp(x + 1e-6)
USE_POOL_USCALE = False

_NC_CACHE = None


def _build_nc():
    import concourse.tile as tile
    from concourse import bacc, mybir

    f32 = mybir.dt.float32
    f32r = mybir.dt.float32r
    AF = mybir.ActivationFunctionType
    MUL = mybir.AluOpType.mult

    nc = bacc.Bacc()
    lg = nc.dram_tensor("lg", [BPC, A, T], f32, kind="ExternalInput")
    # identity for PE transposes; float32r streams at 1.5 cycles/row with a
    # bit-identical data path (same 4-byte elements)
    ident = nc.dram_tensor("ident", [128, 128], f32, kind="ExternalInput")
    # sel[p, b*128+m] = 1.0 if p == b else 0: selector weights that turn the
    # [C,512] v-row tile into a per-sample [128,512] broadcast via PE.
    sel = nc.dram_tensor("sel", [C, C * 128], f32r, kind="ExternalInput")
    out = nc.dram_tensor("out", [BPC, A, T], f32, kind="ExternalOutput")

    G = BPC // C   # number of cohorts
    NP = G // 2    # number of cohort pairs
    SLAB = C * 512  # free elems per slab: per sample 2 halves x 256
    HC = C // 2

    with tile.TileContext(nc) as tc:
        with (
            tc.tile_pool(name="e0p", bufs=8) as e0p,
            tc.tile_pool(name="etp", bufs=4) as etp,
            tc.tile_pool(name="uvp", bufs=8) as uvp,
            tc.tile_pool(name="uvrowp", bufs=3) as uvrowp,
            tc.tile_pool(name="constp", bufs=1) as constp,
            tc.tile_pool(name="psbig", bufs=3, space="PSUM") as psbig,
            tc.tile_pool(name="psuv", bufs=4, space="PSUM") as psuv,
            tc.tile_pool(name="psrow", bufs=1, space="PSUM") as psrow,
        ):
            ident_sb = constp.tile([128, 128], f32)
            nc.sync.dma_start(ident_sb[:], ident[:])
            sel_sb = constp.tile([C, C * 128], f32r)
            nc.sync.dma_start(sel_sb[0:C, :], sel[:])
            v_ones = constp.tile([128, 2 * C], f32)
            nc.vector.memset(v_ones[:], 1.0)
            eps_col = constp.tile([1, 128], f32)
            nc.vector.memset(eps_col[:], EPS)
            ones_row = constp.tile([1, 2 * C], f32)
            nc.vector.memset(ones_row[:], 1.0)

            def load_cohort(g):
                # two half-cohort DMAs so downstream prep can start earlier
                e0 = e0p.tile([128, SLAB], f32, name="e0")
                for h in range(2):
                    src = lg[g * C + h * HC:g * C + (h + 1) * HC].rearrange(
                        "b (h p) j -> p b h j", p=128)
                    nc.sync.dma_start(
                        e0[:, h * HC * 512:(h + 1) * HC * 512].rearrange(
                            "p (b h j) -> p b h j", h=2, j=256), src)
                return e0

            def exp_transpose(e0):
                for h in range(2):
                    sl = slice(h * HC * 512, (h + 1) * HC * 512)
                    nc.scalar.activation(e0[:, sl], e0[:, sl], AF.Exp)
                et = etp.tile([128, SLAB], f32, name="et")
                for b in range(C):
                    pst = psbig.tile([128, 512], f32, name="psbig")
                    for ia in range(2):
                        for jt in range(2):
                            nc.tensor.transpose(
                                pst[:, jt * 256 + ia * 128:
                                    jt * 256 + ia * 128 + 128],
                                e0[:, b * 512 + ia * 256 + jt * 128:
                                   b * 512 + ia * 256 + jt * 128 + 128],
                                ident_sb[:],
                            )
                    nc.scalar.activation(
                        et[:, b * 512:(b + 1) * 512], pst[:], AF.Copy)
                return et

            def phase(st, kind):
                # one Sinkhorn half-iteration: PSUM seeded with eps by a
                # rank-1 PE matmul, 4 accumulating matvecs per sample, then a
                # single DVE reciprocal straight out of PSUM
                ps = psuv.tile([128, 2 * C], f32, name="ps_uv")
                nc.tensor.matmul(
                    ps[:], lhsT=eps_col[0:1, :], rhs=ones_row[0:1, :],
                    start=True, stop=False,
                )
                if kind == "u":
                    mat, vec = st["et"], st["v"]
                else:
                    mat, vec = st["e0"], st["u"]
                for b in range(C):
                    for o in range(2):   # output half (ia for u, jt for v)
                        for c in range(2):  # contraction half
                            # block (o, c) of the stationary matrix: both
                            # layouts put the contraction half at stride 256
                            off = b * 512 + c * 256 + o * 128
                            nc.tensor.matmul(
                                ps[:, o * C + b: o * C + b + 1],
                                lhsT=mat[:, off:off + 128],
                                rhs=vec[:, c * C + b: c * C + b + 1],
                                start=False, stop=(c == 1),
                                skip_group_check=True,
                            )
                cur = uvp.tile([128, 2 * C], f32, name="uv")
                nc.vector.reciprocal(cur[:], ps[:])
                st["u" if kind == "u" else "v"] = cur

            def vrow_stage(st):
                ps_row = psrow.tile([C, 256], f32, name="ps_row")
                for half in range(2):
                    nc.tensor.transpose(
                        ps_row[0:C, half * 128:(half + 1) * 128],
                        st["v"][:, half * C:(half + 1) * C],
                        ident_sb[:],
                    )
                vrow2 = uvrowp.tile([C, 512], f32r, name="vrow2")
                for k in range(2):
                    nc.scalar.activation(
                        vrow2[0:C, k * 256:(k + 1) * 256], ps_row[0:C, :],
                        AF.Copy, scale=OUT_SCALE)
                st["vrow2"] = vrow2

            def final_sample(st, g, b):
                # out_b = (e0_b * u_b[i]) * (v_b[j] * OUT_SCALE), in place
                e0 = st["e0"]
                ps_b = psbig.tile([128, 512], f32, name="psbig")
                nc.tensor.matmul(
                    ps_b[:], lhsT=sel_sb[0:C, b * 128:(b + 1) * 128],
                    rhs=st["vrow2"][0:C, :], start=True, stop=True,
                )
                e0f = e0
                if USE_POOL_USCALE:
                    for ia in range(2):
                        sl = slice(b * 512 + ia * 256, b * 512 + (ia + 1) * 256)
                        nc.gpsimd.tensor_scalar_mul(
                            e0f[:, sl], e0f[:, sl],
                            st["u"][:, ia * C + b: ia * C + b + 1])
                    sl = slice(b * 512, (b + 1) * 512)
                    nc.vector.tensor_mul(e0f[:, sl], e0f[:, sl], ps_b[:])
                else:
                    for ia in range(2):
                        sl = slice(b * 512 + ia * 256, b * 512 + (ia + 1) * 256)
                        nc.vector.scalar_tensor_tensor(
                            e0f[:, sl], e0f[:, sl],
                            st["u"][:, ia * C + b: ia * C + b + 1],
                            ps_b[:, ia * 256:(ia + 1) * 256],
                            MUL, MUL,
                        )
                if b % 4 == 3:
                    q0 = b - 3
                    dst = out[g * C + q0:g * C + b + 1].rearrange(
                        "b (h p) j -> p b h j", p=128)
                    nc.sync.dma_start(
                        dst,
                        e0f[:, q0 * 512:(b + 1) * 512].rearrange(
                            "p (b h j) -> p b h j", h=2, j=256))

            def make_pair(p):
                return [{"v": v_ones, "u": None,
                         "e0": load_cohort(2 * p + gi)} for gi in range(2)]

            def emit_final_steps(fin, steps):
                # fin = (pair_index, states) whose 16 samples are spread
                # across `steps` emission slots
                if fin is None:
                    return iter(())
                p_idx, states = fin
                def gen():
                    for b in range(C):
                        for gi, st in enumerate(states):
                            yield (st, 2 * p_idx + gi, b)
                return gen()

            # ---- software-pipelined cycle loop ----
            loaded = [make_pair(0), make_pair(1)]
            for st in loaded[0]:
                st["et"] = exp_transpose(st["e0"])
            finishing = None
            for p in range(NP):
                states = loaded.pop(0)
                if p + 2 < NP:
                    loaded.append(make_pair(p + 2))
                if loaded:
                    for st in loaded[0]:
                        st["et"] = exp_transpose(st["e0"])
                fin_iter = emit_final_steps(finishing, 10)
                for it in range(ITERS):
                    for kind in ("u", "v"):
                        for st in states:
                            phase(st, kind)
                        # two finishing samples of pair p-1 per phase step
                        for _ in range(2):
                            nxt = next(fin_iter, None)
                            if nxt is not None:
                                final_sample(*nxt)
                for nxt in fin_iter:
                    final_sample(*nxt)
                for st in states:
                    vrow_stage(st)
                finishing = (p, states)
            for nxt in emit_final_steps(finishing, 1):
                final_sample(*nxt)

    nc.compile()
    return nc


def _get_nc():
    global _NC_CACHE
    if _NC_CACHE is None:
        _NC_CACHE = _build_nc()
    return _NC_CACHE


def _prep_in_maps(logits, free_agents_num, tasks_num):
    logits = np.asarray(logits, dtype=np.float32)
    free = np.asarray(free_agents_num).astype(np.int64)
    tasks = np.asarray(tasks_num).astype(np.int64)
    row_ok = np.arange(A, dtype=np.int64)[None, :] < free[:, None]   # [B, A]
    col_ok = np.arange(T, dtype=np.int64)[None, :] < tasks[:, None]  # [B, T]
    mask = row_ok[:, :, None] & col_ok[:, None, :]
    lgm = np.where(mask, logits, MASKVAL).astype(np.float32)
    ident = np.eye(128, dtype=np.float32)
    sel = np.zeros((C, C * 128), dtype=np.float32)
    for b in range(C):
        sel[b, b * 128:(b + 1) * 128] = 1.0
    return [
        {
            "lg": np.ascontiguousarray(lgm[c * BPC:(c + 1) * BPC]),
            "ident": ident,
            "sel": sel,
        }
        for c in range(NCORES)
    ]


def _run(logits, free_agents_num, tasks_num, **spmd_kwargs):
    from concourse.bass_utils import run_bass_kernel_spmd

    in_maps = _prep_in_maps(logits, free_agents_num, tasks_num)
    res = run_bass_kernel_spmd(
        _get_nc(), in_maps, core_ids=list(range(NCORES)), **spmd_kwargs
    )
    out = np.concatenate([r["out"] for r in res.results], axis=0)
    return out, res


def kernel(logits, free_agents_num, tasks_num):
    out, _ = _run(logits, free_agents_num, tasks_num)
    return out
